# revision 3
# baseline (speedup 1.0000x reference)
"""Trainium2 Bass kernel for nn_CrossPairMemory.

Sharding: data-parallel over batch across 8 NeuronCores (512 rows each),
weights replicated per core, no collectives.  All heavy matmuls run in
bf16 (fp32 PSUM accumulation); LayerNorm statistics and normalization in
fp32.  Activations are kept transposed (features on partitions, batch on
the free axis) through the fusion MLP so weight tiles act as the
stationary matmul operand in their natural HBM layout; the final
per-pair stage flips to activations-stationary so the output psum is
row-major and the last LayerNorm reduces along the free axis.
"""

import sys

for _p in ("/opt/trn_rl_repo",):
    if _p not in sys.path:
        sys.path.insert(0, _p)

import numpy as np
import ml_dtypes

import concourse.bass as bass
import concourse.tile as tile
from concourse import bacc, mybir
from concourse import bass_utils

BF = ml_dtypes.bfloat16
dt = mybir.dt
AF = mybir.ActivationFunctionType
ALU = mybir.AluOpType

NCORES = 8
B, P, PD, MD, S = 4096, 28, 128, 256, 64
D = P * PD            # 3584
K1T = 2 * P           # 56 contraction tiles for the first fusion matmul
Bc = B // NCORES      # 512 batch rows per core
# batch sub-chunks inside a core: small first chunk so its LN/gelu pass
# overlaps the second chunk's matmuls on the PE.
CHUNKS = ((0, 128), (128, 384))
EPS = 1e-5


def _bcast_ap(src_row):
    """Replicate a [N]-shaped dram AP across 128 partitions (stride-0)."""
    return bass.AP(
        tensor=src_row.tensor,
        offset=src_row.offset,
        ap=[[0, PD]] + [list(x) for x in src_row.ap],
    )


def _build():
    nc = bacc.Bacc(
        "TRN2", target_bir_lowering=False, debug=False, num_devices=NCORES
    )

    def din(name, shape, dty):
        return nc.dram_tensor(name, list(shape), dty, kind="ExternalInput").ap()

    psT = din("psT", (P, PD, Bc), dt.bfloat16)      # pair_states^T per pair
    msT = din("msT", (MD, Bc), dt.bfloat16)         # macro_state^T
    kP = din("kP", (PD, S), dt.bfloat16)            # pair keys^T, pre-scaled
    kM = din("kM", (MD, S), dt.bfloat16)            # macro keys^T, pre-scaled
    vP = din("vP", (S, D), dt.bfloat16)
    vM = din("vM", (S, D), dt.bfloat16)
    w1r = din("w1r", (P, PD, K1T, PD), dt.bfloat16)  # [n, kp, kt, f]
    w2r = din("w2r", (P, PD, P, PD), dt.bfloat16)    # [m, kp, kt, f]
    b1t = din("b1t", (PD, P), dt.float32)
    g1t = din("g1t", (PD, P), dt.float32)
    be1t = din("be1t", (PD, P), dt.float32)
    b2t = din("b2t", (PD, P), dt.float32)
    pwr = din("pwr", (PD, P, 2, PD), dt.bfloat16)    # [d, pair, ktile, e]
    pbr = din("pbr", (1, P, PD), dt.bfloat16)
    pgr = din("pgr", (P, PD), dt.float32)
    pber = din("pber", (P, PD), dt.float32)
    out = nc.dram_tensor("out", [Bc, P, PD], dt.float32, kind="ExternalOutput").ap()

    with tile.TileContext(nc) as tc:
        with (
            tc.tile_pool(name="const", bufs=1) as const,
            tc.tile_pool(name="res", bufs=1) as res,
        ):
            ones_col = const.tile([PD, 1], dt.bfloat16, tag="ones_col", name="ones_col")
            nc.vector.memset(ones_col, 1.0)
            ones_row_f = const.tile([1, PD], dt.float32, tag="ones_row_f", name="ones_row_f")
            nc.vector.memset(ones_row_f, 1.0)
            ones_row_b = const.tile([1, PD], dt.bfloat16, tag="ones_row_b", name="ones_row_b")
            nc.vector.memset(ones_row_b, 1.0)
            eps_t = const.tile([PD, 1], dt.float32, tag="eps", name="eps")
            nc.vector.memset(eps_t, EPS)

            lnc = {}
            for nm, src in (("b1", b1t), ("g1", g1t), ("be1", be1t), ("b2", b2t)):
                t = const.tile([PD, P], dt.float32, tag=f"lnc_{nm}", name=f"lnc_{nm}")
                nc.sync.dma_start(t, src)
                lnc[nm] = t
            pw_sb = const.tile([PD, P, 2, PD], dt.bfloat16, tag="pw_sb", name="pw_sb")
            nc.sync.dma_start(pw_sb, pwr)
            pb_sb = const.tile([1, P, PD], dt.bfloat16, tag="pb_sb", name="pb_sb")
            nc.sync.dma_start(pb_sb, pbr)

            # pair_states^T tiles stay resident: used by the score matmuls
            # (stage A) and again as stationary operands in stage C.
            psT_sb = []
            for p in range(P):
                t = res.tile([PD, Bc], dt.bfloat16, tag=f"psT{p}", name=f"psT{p}")
                nc.sync.dma_start(t, psT[p])
                psT_sb.append(t)

            with (
                tc.tile_pool(name="xt", bufs=1) as pxt,
                tc.tile_pool(name="h2", bufs=1) as ph2,
            ):
                xt_sb = [
                    pxt.tile([PD, Bc], dt.bfloat16, tag=f"xt{k}", name=f"xt{k}")
                    for k in range(K1T)
                ]
                h2_sb = [
                    ph2.tile([PD, Bc], dt.bfloat16, tag=f"h2{n}", name=f"h2{n}")
                    for n in range(P)
                ]

                # ---------------- stage A: associative memory reads --------
                with (
                    tc.tile_pool(name="stA", bufs=1) as pa,
                    tc.tile_pool(name="psA", bufs=2, space="PSUM") as ppa,
                    tc.tile_pool(name="psAc", bufs=2, space="PSUM") as ppac,
                ):
                    vP_sb = pa.tile([S, D], dt.bfloat16, tag="vP", name="vP")
                    nc.sync.dma_start(vP_sb, vP)
                    vM_sb = pa.tile([S, D], dt.bfloat16, tag="vM", name="vM")
                    nc.sync.dma_start(vM_sb, vM)
                    kP_sb = pa.tile([PD, S], dt.bfloat16, tag="kP", name="kP")
                    nc.sync.dma_start(kP_sb, kP)
                    kM0 = pa.tile([PD, S], dt.bfloat16, tag="kM0", name="kM0")
                    nc.sync.dma_start(kM0, kM[0:PD])
                    kM1 = pa.tile([PD, S], dt.bfloat16, tag="kM1", name="kM1")
                    nc.sync.dma_start(kM1, kM[PD:MD])
                    ms0 = pa.tile([PD, Bc], dt.bfloat16, tag="ms0", name="ms0")
                    nc.sync.dma_start(ms0, msT[0:PD])
                    ms1 = pa.tile([PD, Bc], dt.bfloat16, tag="ms1", name="ms1")
                    nc.sync.dma_start(ms1, msT[PD:MD])

                    def memory_read(which, vals_sb, xt_off):
                        sp = ppa.tile([S, Bc], dt.float32, tag="sp", name="sp")
                        if which == "pair":
                            for p in range(P):
                                nc.tensor.matmul(
                                    sp, kP_sb, psT_sb[p],
                                    start=(p == 0), stop=(p == P - 1),
                                )
                        else:
                            nc.tensor.matmul(sp, kM0, ms0, start=True, stop=False)
                            nc.tensor.matmul(sp, kM1, ms1, start=False, stop=True)
                        # scores are O(0.3): exp without max-subtraction is safe
                        eb = pa.tile([S, Bc], dt.bfloat16, tag=f"eb_{which}", name=f"eb_{which}")
                        nc.scalar.activation(eb, sp, AF.Exp)
                        den = ppa.tile([1, Bc], dt.float32, tag="den", name="den")
                        nc.tensor.matmul(den, ones_col[0:S, :], eb, start=True, stop=True)
                        rr = pa.tile([1, Bc], dt.float32, tag=f"rr_{which}", name=f"rr_{which}")
                        nc.vector.reciprocal(rr, den)
                        rbc = ppa.tile([S, Bc], dt.float32, tag="rbc", name="rbc")
                        nc.tensor.matmul(
                            rbc, ones_row_f[:, 0:S], rr, start=True, stop=True
                        )
                        ab = pa.tile([S, Bc], dt.bfloat16, tag=f"ab_{which}", name=f"ab_{which}")
                        nc.vector.tensor_mul(ab, eb, rbc)
                        for d in range(P):
                            pc = ppac.tile([PD, Bc], dt.float32, tag="pc", name="pc")
                            nc.tensor.matmul(
                                pc, vals_sb[:, d * PD:(d + 1) * PD], ab,
                                start=True, stop=True,
                            )
                            nc.scalar.activation(xt_sb[xt_off + d], pc, AF.Copy)

                    memory_read("pair", vP_sb, 0)
                    memory_read("macro", vM_sb, P)

                # ---------------- stage B: fusion MLP -----------------------
                with (
                    tc.tile_pool(name="hbf", bufs=1) as phb,
                    tc.tile_pool(name="psStat", bufs=1, space="PSUM") as ppst,
                ):
                    hbf = [
                        phb.tile([PD, Bc], dt.bfloat16, tag=f"hbf{n}", name=f"hbf{n}")
                        for n in range(P)
                    ]
                    stat_h = ppst.tile([1, Bc], dt.float32, tag="stat_h", name="stat_h")
                    stat_q = ppst.tile([1, Bc], dt.float32, tag="stat_q", name="stat_q")

                    with (
                        tc.tile_pool(name="w1s", bufs=2) as pw1,
                        tc.tile_pool(name="sqs", bufs=3) as psq,
                        tc.tile_pool(name="psM1", bufs=2, space="PSUM") as ppm1,
                    ):
                        for n in range(P):
                            w1b = pw1.tile([PD, K1T, PD], dt.bfloat16, tag="w1blk", name="w1blk")
                            nc.sync.dma_start(w1b, w1r[n])
                            for ci, (co, csz) in enumerate(CHUNKS):
                                pm = ppm1.tile([PD, csz], dt.float32, tag=f"pm{ci}", name=f"pm{ci}")
                                for k in range(K1T):
                                    nc.tensor.matmul(
                                        pm, w1b[:, k, :],
                                        xt_sb[k][:, co:co + csz],
                                        start=(k == 0), stop=(k == K1T - 1),
                                    )
                                nc.scalar.activation(
                                    hbf[n][:, co:co + csz], pm, AF.Identity,
                                    bias=lnc["b1"][:, n:n + 1], scale=1.0,
                                )
                            sq = psq.tile([PD, Bc], dt.bfloat16, tag="sq", name="sq")
                            nc.vector.tensor_mul(sq, hbf[n], hbf[n])
                            for co, csz in CHUNKS:
                                nc.tensor.matmul(
                                    stat_h[:, co:co + csz], ones_col,
                                    hbf[n][:, co:co + csz],
                                    start=(n == 0), stop=(n == P - 1),
                                    skip_group_check=True,
                                )
                                nc.tensor.matmul(
                                    stat_q[:, co:co + csz], ones_col,
                                    sq[:, co:co + csz],
                                    start=(n == 0), stop=(n == P - 1),
                                    skip_group_check=True,
                                )

                    # LayerNorm + gelu (per batch chunk)
                    with (
                        tc.tile_pool(name="lnrow", bufs=2) as plr,
                        tc.tile_pool(name="psBC", bufs=1, space="PSUM") as ppbc,
                        tc.tile_pool(name="tnorm", bufs=3) as ptn,
                    ):
                        for ci, (co, csz) in enumerate(CHUNKS):
                            cs = slice(co, co + csz)
                            mu_row = plr.tile([1, csz], dt.float32, tag=f"mu{ci}", name=f"mu{ci}")
                            nc.scalar.activation(
                                mu_row, stat_h[:, cs], AF.Copy, scale=1.0 / D
                            )
                            m2_row = plr.tile([1, csz], dt.float32, tag=f"m2{ci}", name=f"m2{ci}")
                            nc.scalar.activation(
                                m2_row, stat_q[:, cs], AF.Copy, scale=1.0 / D
                            )
                            var_row = plr.tile([1, csz], dt.float32, tag=f"va{ci}", name=f"va{ci}")
                            nc.vector.tensor_mul(var_row, mu_row, mu_row)
                            nc.vector.tensor_sub(var_row, m2_row, var_row)
                            sd_row = plr.tile([1, csz], dt.float32, tag=f"sd{ci}", name=f"sd{ci}")
                            nc.scalar.activation(
                                sd_row, var_row, AF.Sqrt,
                                bias=eps_t[0:1, :], scale=1.0,
                            )
                            rstd_row = plr.tile([1, csz], dt.float32, tag=f"rs{ci}", name=f"rs{ci}")
                            nc.vector.reciprocal(rstd_row, sd_row)
                            mu_bc = ppbc.tile([PD, csz], dt.float32, tag=f"mubc{ci}", name=f"mubc{ci}")
                            nc.tensor.matmul(
                                mu_bc, ones_row_f, mu_row, start=True, stop=True
                            )
                            rs_bc = ppbc.tile([PD, csz], dt.float32, tag=f"rsbc{ci}", name=f"rsbc{ci}")
                            nc.tensor.matmul(
                                rs_bc, ones_row_f, rstd_row, start=True, stop=True
                            )
                            for n in range(P):
                                t1 = ptn.tile([PD, csz], dt.float32, tag=f"t1_{ci}", name=f"t1_{ci}")
                                nc.vector.scalar_tensor_tensor(
                                    t1, hbf[n][:, cs], 1.0, mu_bc,
                                    op0=ALU.mult, op1=ALU.subtract,
                                )
                                t2 = ptn.tile([PD, csz], dt.float32, tag=f"t2_{ci}", name=f"t2_{ci}")
                                nc.vector.scalar_tensor_tensor(
                                    t2, t1, lnc["g1"][:, n:n + 1], rs_bc,
                                    op0=ALU.mult, op1=ALU.mult,
                                )
                                nc.scalar.activation(
                                    h2_sb[n][:, cs], t2, AF.Gelu,
                                    bias=lnc["be1"][:, n:n + 1], scale=1.0,
                                )

                # ------------- stage B2 + C: second matmul & per-pair -------
                with (
                    tc.tile_pool(name="w2s", bufs=2) as pw2,
                    tc.tile_pool(name="fus", bufs=3) as pfu,
                    tc.tile_pool(name="cbc", bufs=4) as pcb,
                    tc.tile_pool(name="scm", bufs=4) as psc,
                    tc.tile_pool(name="yout", bufs=3) as pyo,
                    tc.tile_pool(name="psM2", bufs=2, space="PSUM") as ppm2,
                    tc.tile_pool(name="psC", bufs=3, space="PSUM") as ppc,
                ):
                    for ci, (co, csz) in enumerate(CHUNKS):
                        cs = slice(co, co + csz)
                        for m in range(P):
                            w2b = pw2.tile([PD, P, PD], dt.bfloat16, tag="w2blk", name="w2blk")
                            nc.sync.dma_start(w2b, w2r[m])
                            pf = ppm2.tile([PD, csz], dt.float32, tag=f"pf{ci}", name=f"pf{ci}")
                            for k in range(P):
                                nc.tensor.matmul(
                                    pf, w2b[:, k, :], h2_sb[k][:, cs],
                                    start=(k == 0), stop=(k == P - 1),
                                )
                            fz = pfu.tile([PD, csz], dt.bfloat16, tag=f"fz{ci}", name=f"fz{ci}")
                            nc.scalar.activation(
                                fz, pf, AF.Identity,
                                bias=lnc["b2"][:, m:m + 1], scale=1.0,
                            )
                            gb = pcb.tile([PD, PD], dt.float32, tag="gbc", name="gbc")
                            nc.sync.dma_start(gb, _bcast_ap(pgr[m]))
                            bb = pcb.tile([PD, PD], dt.float32, tag="bbc", name="bbc")
                            nc.sync.dma_start(bb, _bcast_ap(pber[m]))
                            for bt in range(csz // PD):
                                bs = slice(co + bt * PD, co + (bt + 1) * PD)
                                po = ppc.tile([PD, PD], dt.float32, tag="po", name="po")
                                nc.tensor.matmul(
                                    po, psT_sb[m][:, bs], pw_sb[:, m, 0, :],
                                    start=True, stop=False,
                                )
                                nc.tensor.matmul(
                                    po, fz[:, bt * PD:(bt + 1) * PD],
                                    pw_sb[:, m, 1, :],
                                    start=False, stop=False,
                                )
                                nc.tensor.matmul(
                                    po, ones_row_b, pb_sb[:, m, :],
                                    start=False, stop=True,
                                )
                                st6 = psc.tile([PD, 6], dt.float32, tag="st6", name="st6")
                                nc.vector.bn_stats(st6, po)
                                mv = psc.tile([PD, 2], dt.float32, tag="mv", name="mv")
                                nc.vector.bn_aggr(mv, st6)
                                sd2 = psc.tile([PD, 1], dt.float32, tag="sd2", name="sd2")
                                nc.scalar.activation(
                                    sd2, mv[:, 1:2], AF.Sqrt,
                                    bias=eps_t, scale=1.0,
                                )
                                rst2 = psc.tile([PD, 1], dt.float32, tag="rst2", name="rst2")
                                nc.vector.reciprocal(rst2, sd2)
                                tn = pyo.tile([PD, PD], dt.float32, tag="tn", name="tn")
                                nc.vector.tensor_scalar(
                                    tn, po, mv[:, 0:1], rst2,
                                    op0=ALU.subtract, op1=ALU.mult,
                                )
                                nc.vector.tensor_mul(tn, tn, gb)
                                y = pyo.tile([PD, PD], dt.float32, tag="y", name="y")
                                nc.vector.tensor_add(y, tn, bb)
                                nc.sync.dma_start(out[bs, m, :], y)

    nc.compile()
    return nc


_CACHE = {}


def _get_nc():
    if "nc" not in _CACHE:
        _CACHE["nc"] = _build()
    return _CACHE["nc"]


def _prep_in_maps(inputs):
    f32 = np.float32
    g = lambda k: np.asarray(inputs[k], f32)

    psT_full = np.asarray(g("pair_states").transpose(1, 2, 0), dtype=BF)   # [P,PD,B]
    msT_full = np.asarray(g("macro_state").T, dtype=BF)                    # [MD,B]

    shared = {
        "kP": np.ascontiguousarray(
            (g("mem_pair_keys").T / (P * np.sqrt(PD))).astype(BF)),
        "kM": np.ascontiguousarray(
            (g("mem_macro_keys").T / np.sqrt(MD)).astype(BF)),
        "vP": g("mem_pair_vals").astype(BF),
        "vM": g("mem_macro_vals").astype(BF),
        "w1r": np.ascontiguousarray(
            g("fusion_w1").reshape(K1T, PD, P, PD).transpose(2, 1, 0, 3)
        ).astype(BF),
        "w2r": np.ascontiguousarray(
            g("fusion_w2").reshape(P, PD, P, PD).transpose(2, 1, 0, 3)
        ).astype(BF),
        "b1t": np.ascontiguousarray(g("fusion_b1").reshape(P, PD).T),
        "g1t": np.ascontiguousarray(g("fusion_ln_g").reshape(P, PD).T),
        "be1t": np.ascontiguousarray(g("fusion_ln_b").reshape(P, PD).T),
        "b2t": np.ascontiguousarray(g("fusion_b2").reshape(P, PD).T),
        "pwr": np.ascontiguousarray(
            g("pair_w").reshape(P, 2, PD, PD).transpose(2, 0, 1, 3)
        ).astype(BF),
        "pbr": g("pair_b").astype(BF).reshape(1, P, PD),
        "pgr": np.ascontiguousarray(g("pair_ln_g")),
        "pber": np.ascontiguousarray(g("pair_ln_b")),
    }
    in_maps = []
    for c in range(NCORES):
        m = dict(shared)
        m["psT"] = np.ascontiguousarray(psT_full[:, :, c * Bc:(c + 1) * Bc])
        m["msT"] = np.ascontiguousarray(msT_full[:, c * Bc:(c + 1) * Bc])
        in_maps.append(m)
    return in_maps


def _run(inputs, trace=False):
    nc = _get_nc()
    in_maps = _prep_in_maps(inputs)
    res = bass_utils.run_bass_kernel_spmd(
        nc, in_maps, core_ids=list(range(NCORES)), trace=trace
    )
    outp = np.concatenate([res.results[c]["out"] for c in range(NCORES)], axis=0)
    return np.ascontiguousarray(outp.astype(np.float32)), res


def kernel(**inputs):
    outp, _ = _run(inputs, trace=False)
    return outp
